# revision 17
# baseline (speedup 1.0000x reference)
"""Sequence-parallel single-head attention block (LN -> QKV -> softmax(QK^T)V -> proj -> residual)
for 8 Trainium2 NeuronCores.

Sharding: core i owns query rows [1024*i, 1024*(i+1)). K/V projections over all 8192
tokens are replicated on every core (no collectives). Inside each core a flash-style
loop streams 512-key chunks: LN + K/V projection + scores + exp + AV accumulate,
with scores held transposed (keys on partitions) so that
  - exp() is the PSUM->SBUF eviction on the scalar engine,
  - softmax denominators come from a ones-vector matmul on the tensor engine,
  - P^T and V feed the AV matmul directly (no P transposes).

Host-side exact algebra folds:
  - ln_w/ln_b fold into the QKV weight matrices / biases,
  - the 1/sqrt(c) score scale folds into Wq/bq,
  - bk drops exactly (softmax is invariant to per-query score shifts),
  - bv folds into the output-projection bias (P @ 1 = denom), bp' = bp + Wp @ bv,
  - softmax runs without max subtraction (scores bounded ~|2| for these inputs).

Matmuls run in float32r (full PE rate); operands are rounded to fp32r for free inside
the DVE/ACT evictions that produce them.
"""

import math
from contextlib import ExitStack

import numpy as np

import concourse.bass as bass
import concourse.bacc as bacc
import concourse.tile as tile
from concourse import mybir
from concourse.bass_utils import run_bass_kernel_spmd
from concourse.masks import make_identity

N, NF = 8192, 512
NCORES = 8
BLK = N // NCORES          # 1024 query rows per core
MC = 512                   # key-chunk size
NCHUNK = N // MC           # 16
EPS = 1e-5

F32 = mybir.dt.float32
F32R = mybir.dt.float32r
AF = mybir.ActivationFunctionType

TRACE = False              # test.py flips this for timed runs
LAST_EXEC_NS = None

_cached_nc = None


def _build():
    nc = bacc.Bacc("TRN2", target_bir_lowering=False, debug=False)

    x_all = nc.dram_tensor("x_all", [N, NF], F32, kind="ExternalInput")
    xq = nc.dram_tensor("xq", [BLK, NF], F32, kind="ExternalInput")
    wqt = nc.dram_tensor("wqt", [NF, NF], F32, kind="ExternalInput")  # (Wq*ln_w).T * scale
    wkt = nc.dram_tensor("wkt", [NF, NF], F32, kind="ExternalInput")  # (Wk*ln_w).T
    wvt = nc.dram_tensor("wvt", [NF, NF], F32, kind="ExternalInput")  # (Wv*ln_w).T
    wpt = nc.dram_tensor("wpt", [NF, NF], F32, kind="ExternalInput")  # Wp.T
    bqs = nc.dram_tensor("bqs", [NF], F32, kind="ExternalInput")      # (bq + Wq@ln_b)*scale
    bp2 = nc.dram_tensor("bp2", [NF], F32, kind="ExternalInput")      # bp + Wp@bv
    y_out = nc.dram_tensor("y", [BLK, NF], F32, kind="ExternalOutput")

    with tile.TileContext(nc) as tc, ExitStack() as ctx:
        # ---- pools ----
        const = ctx.enter_context(tc.tile_pool(name="const", bufs=1))
        wpool = ctx.enter_context(tc.tile_pool(name="wpool", bufs=1))
        xcp = ctx.enter_context(tc.tile_pool(name="xcp", bufs=2))
        xtp = ctx.enter_context(tc.tile_pool(name="xtp", bufs=2))
        ktp = ctx.enter_context(tc.tile_pool(name="ktp", bufs=2))
        vp = ctx.enter_context(tc.tile_pool(name="vp", bufs=2))
        ptp = ctx.enter_context(tc.tile_pool(name="ptp", bufs=2))
        stat = ctx.enter_context(tc.tile_pool(name="stat", bufs=4))
        acc = ctx.enter_context(tc.tile_pool(name="acc", bufs=1))
        yp = ctx.enter_context(tc.tile_pool(name="yp", bufs=2))
        xop = ctx.enter_context(tc.tile_pool(name="xop", bufs=1))
        ps = ctx.enter_context(tc.tile_pool(name="ps", bufs=4, space="PSUM"))
        psav = ctx.enter_context(tc.tile_pool(name="psav", bufs=2, space="PSUM"))
        psd = ctx.enter_context(tc.tile_pool(name="psd", bufs=1, space="PSUM"))

        # ---- constants / weights ----
        ident_f = const.tile([128, 128], F32, tag="ident_f")
        make_identity(nc, ident_f[:])
        ident = const.tile([128, 128], F32R, tag="ident")
        nc.vector.tensor_copy(out=ident[:], in_=ident_f[:])
        ones_f = const.tile([128, MC], F32, tag="ones_f")
        nc.vector.memset(ones_f[:], 1.0)
        ones_col = const.tile([128, 1], F32R, tag="ones_col")
        nc.vector.tensor_copy(out=ones_col[:], in_=ones_f[:, 0:1])
        ones_row = const.tile([1, MC], F32R, tag="ones_row")
        nc.vector.tensor_copy(out=ones_row[:], in_=ones_f[0:1, :])
        eps_t = const.tile([128, 1], F32, tag="eps")
        nc.vector.memset(eps_t[:], EPS)
        ones11 = const.tile([1, 1], F32, tag="ones11")
        nc.vector.memset(ones11[:], 1.0)

        w_sb = {}
        for name, drm in (("wq", wqt), ("wk", wkt), ("wv", wvt), ("wp", wpt)):
            t = wpool.tile([128, 4, NF], F32R, tag=name)
            nc.gpsimd.dma_start(
                out=t[:], in_=drm.ap().rearrange("(s p) e -> p s e", p=128)
            )
            w_sb[name] = t
        bq_sb = const.tile([1, NF], F32R, tag="bq")
        nc.gpsimd.dma_start(out=bq_sb[:], in_=bqs.ap().rearrange("(o e) -> o e", o=1))
        bp2_sb = const.tile([128, NF], F32, tag="bp2")
        bp2_b = bass.AP(tensor=bp2.ap().tensor, offset=bp2.ap().offset,
                        ap=[[0, 128]] + bp2.ap().ap)
        nc.gpsimd.dma_start(out=bp2_sb[:], in_=bp2_b)

        qt_sb = acc.tile([128, 4, BLK], F32R, tag="qt")       # q^T [c, n]
        at_sb = acc.tile([128, 4, BLK], F32, tag="attnT")     # attn^T accumulator [c, n]
        den_sb = acc.tile([1, BLK], F32, tag="den")
        rd_sb = acc.tile([128, BLK // 128], F32, tag="rd")

        def ln_chunk(src_dram, m0):
            """Load 512 rows starting at m0 (cast to f32r in-DMA), LayerNorm-normalize
            in place. Returns [128, 4, 512] f32r (m within block, block, d)."""
            xc = xcp.tile([128, 4, NF], F32R, tag="xc")
            nc.gpsimd.dma_start(
                out=xc[:],
                in_=src_dram.ap()[m0:m0 + MC, :].rearrange("(t p) d -> p t d", p=128),
            )
            for t in range(4):
                st = stat.tile([128, 6], F32, tag="st")
                mv = stat.tile([128, 2], F32, tag="mv")
                rstd = stat.tile([128, 1], F32, tag="rstd")
                nc.vector.bn_stats(out=st[:], in_=xc[:, t, :])
                nc.vector.bn_aggr(out=mv[:], in_=st[:])
                # rstd = exp(-0.5*ln(var+eps)); ln+exp share one ACT table set
                nc.scalar.activation(out=rstd[:], in_=mv[:, 1:2], func=AF.Ln,
                                     bias=eps_t[:], scale=1.0)
                nc.scalar.activation(out=rstd[:], in_=rstd[:], func=AF.Exp,
                                     scale=-0.5)
                nc.vector.tensor_scalar(
                    out=xc[:, t, :], in0=xc[:, t, :],
                    scalar1=mv[:, 0:1], scalar2=rstd[:],
                    op0=mybir.AluOpType.subtract, op1=mybir.AluOpType.mult,
                )
            return xc

        def transpose_chunk(xc):
            """[128,4,512] normalized chunk -> x^T chunk [128, 4, 512] = (d, dslice, m)."""
            xt = xtp.tile([128, 4, MC], F32R, tag="xt")
            for ds in range(4):
                ptile = ps.tile([128, MC], F32R, tag="ps")
                for t in range(4):
                    nc.tensor.transpose(
                        ptile[:, t * 128:(t + 1) * 128],
                        xc[:, t, ds * 128:(ds + 1) * 128],
                        ident[:],
                    )
                nc.scalar.activation(out=xt[:, ds, :], in_=ptile[:], func=AF.Copy)
            return xt

        # ---- Phase A: q^T for this core's 1024 rows ----
        for oc in range(BLK // MC):
            xc = ln_chunk(xq, oc * MC)
            xt = transpose_chunk(xc)
            for cc in range(4):
                ptile = ps.tile([128, MC], F32, tag="ps")
                for ds in range(4):
                    nc.tensor.matmul(
                        ptile[:], w_sb["wq"][:, ds, cc * 128:(cc + 1) * 128],
                        xt[:, ds, :], start=(ds == 0), stop=False,
                    )
                nc.tensor.matmul(
                    ptile[:], bq_sb[:, cc * 128:(cc + 1) * 128], ones_row[:],
                    start=False, stop=True,
                )
                nc.scalar.activation(out=qt_sb[:, cc, oc * MC:(oc + 1) * MC],
                                     in_=ptile[:], func=AF.Copy)

        # ---- persistent denominator PSUM tiles ----
        pd = []
        for nh in range(2):
            pd_t = psd.tile([1, MC], F32, tag=f"d{nh}")
            pd.append(pd_t)

        # ---- Phase B: stream key chunks ----
        for ch in range(NCHUNK):
            xc = ln_chunk(x_all, ch * MC)
            xt = transpose_chunk(xc)

            # k^T chunk [c, m]
            kt = ktp.tile([128, 4, MC], F32R, tag="kt")
            for cc in range(4):
                ptile = ps.tile([128, MC], F32, tag="ps")
                for ds in range(4):
                    nc.tensor.matmul(
                        ptile[:], w_sb["wk"][:, ds, cc * 128:(cc + 1) * 128],
                        xt[:, ds, :], start=(ds == 0), stop=(ds == 3),
                    )
                nc.scalar.activation(out=kt[:, cc, :], in_=ptile[:], func=AF.Copy)

            # v chunk [m, c]
            vt = vp.tile([128, 4, NF], F32R, tag="vt")
            for mb in range(4):
                ptile = ps.tile([128, NF], F32, tag="ps")
                for ds in range(4):
                    nc.tensor.matmul(
                        ptile[:], xt[:, ds, mb * 128:(mb + 1) * 128],
                        w_sb["wv"][:, ds, :], start=(ds == 0), stop=(ds == 3),
                    )
                nc.scalar.activation(out=vt[:, mb, :], in_=ptile[:], func=AF.Copy)

            # scores^T -> exp -> P^T  [m, n]
            pt = ptp.tile([128, 4, BLK], F32R, tag="pt")
            for mb in range(4):
                for nh in range(2):
                    ptile = ps.tile([128, MC], F32, tag="ps")
                    for cc in range(4):
                        nc.tensor.matmul(
                            ptile[:], kt[:, cc, mb * 128:(mb + 1) * 128],
                            qt_sb[:, cc, nh * 512:(nh + 1) * 512],
                            start=(cc == 0), stop=(cc == 3),
                        )
                    nc.scalar.activation(
                        out=pt[:, mb, nh * 512:(nh + 1) * 512], in_=ptile[:],
                        func=AF.Exp,
                    )

            # denominators: ones^T @ P^T accumulated across all chunks
            for mb in range(4):
                for nh in range(2):
                    nc.tensor.matmul(
                        pd[nh][:], ones_col[:],
                        pt[:, mb, nh * 512:(nh + 1) * 512],
                        start=(ch == 0 and mb == 0), stop=(ch == NCHUNK - 1 and mb == 3),
                        skip_group_check=True,
                    )

            # attn^T partial: v^T-chunks as stationary, P^T moving
            for cc in range(4):
                for nh in range(2):
                    av = psav.tile([128, MC], F32, tag="av")
                    for mb in range(4):
                        nc.tensor.matmul(
                            av[:], vt[:, mb, cc * 128:(cc + 1) * 128],
                            pt[:, mb, nh * 512:(nh + 1) * 512],
                            start=(mb == 0), stop=(mb == 3),
                        )
                    dst = at_sb[:, cc, nh * 512:(nh + 1) * 512]
                    if ch == 0:
                        nc.vector.tensor_copy(out=dst, in_=av[:])
                    else:
                        nc.vector.tensor_tensor(
                            out=dst, in0=dst, in1=av[:], op=mybir.AluOpType.add,
                        )

        # ---- epilogue ----
        for nh in range(2):
            nc.vector.tensor_copy(out=den_sb[:, nh * 512:(nh + 1) * 512], in_=pd[nh][:])
        # transpose the denominator row into partitions: [1,128] x [1,1] matmuls
        prd = ps.tile([128, BLK // 128], F32, tag="ps")
        for j in range(BLK // 128):
            nc.tensor.matmul(prd[:, j:j + 1], den_sb[:, j * 128:(j + 1) * 128],
                             ones11[:], start=True, stop=True,
                             skip_group_check=True)
        nc.vector.reciprocal(out=rd_sb[:], in_=prd[:])

        # residual rows + folded bias
        xo_tiles = []
        for j in range(BLK // 128):
            xo = xop.tile([128, NF], F32, tag=f"xo{j}")
            nc.sync.dma_start(out=xo[:], in_=xq.ap()[j * 128:(j + 1) * 128, :])
            nc.vector.tensor_tensor(out=xo[:], in0=xo[:], in1=bp2_sb[:],
                                    op=mybir.AluOpType.add)
            xo_tiles.append(xo)

        at_rt = ptp.tile([128, 4, BLK], F32R, tag="pt")
        nc.vector.tensor_copy(out=at_rt[:], in_=at_sb[:])
        at_r = at_rt[:]
        for j in range(BLK // 128):
            ptile = ps.tile([128, NF], F32, tag="ps")
            for cc in range(4):
                nc.tensor.matmul(
                    ptile[:], at_r[:, cc, j * 128:(j + 1) * 128],
                    w_sb["wp"][:, cc, :], start=(cc == 0), stop=(cc == 3),
                )
            yt = yp.tile([128, NF], F32, tag="yt")
            nc.vector.tensor_scalar_mul(out=yt[:], in0=ptile[:],
                                        scalar1=rd_sb[:, j:j + 1])
            nc.vector.tensor_tensor(out=yt[:], in0=yt[:], in1=xo_tiles[j][:],
                                    op=mybir.AluOpType.add)
            nc.sync.dma_start(out=y_out.ap()[j * 128:(j + 1) * 128, :], in_=yt[:])

    nc.compile()
    return nc


def kernel(x, ln_w, ln_b, Wq, bq, Wk, bk, Wv, bv, Wp, bp):
    global _cached_nc, LAST_EXEC_NS
    x = np.ascontiguousarray(np.asarray(x, dtype=np.float32))
    ln_w = np.asarray(ln_w, np.float32)
    ln_b = np.asarray(ln_b, np.float32)
    Wq = np.asarray(Wq, np.float32)
    Wk = np.asarray(Wk, np.float32)
    Wv = np.asarray(Wv, np.float32)
    Wp = np.asarray(Wp, np.float32)
    scale = np.float32(1.0 / math.sqrt(NF))

    # exact algebraic folds (see module docstring)
    wqt = np.ascontiguousarray(((Wq * ln_w[None, :]).T * scale).astype(np.float32))
    wkt = np.ascontiguousarray((Wk * ln_w[None, :]).T.astype(np.float32))
    wvt = np.ascontiguousarray((Wv * ln_w[None, :]).T.astype(np.float32))
    wpt = np.ascontiguousarray(Wp.T.astype(np.float32))
    bqs = ((np.asarray(bq, np.float32) + Wq @ ln_b) * scale).astype(np.float32)
    bp2 = (np.asarray(bp, np.float32) + Wp @ np.asarray(bv, np.float32)).astype(np.float32)

    if _cached_nc is None:
        _cached_nc = _build()
    nc = _cached_nc

    in_maps = []
    for i in range(NCORES):
        in_maps.append({
            "x_all": x,
            "xq": np.ascontiguousarray(x[i * BLK:(i + 1) * BLK]),
            "wqt": wqt, "wkt": wkt, "wvt": wvt, "wpt": wpt,
            "bqs": bqs, "bp2": bp2,
        })
    res = run_bass_kernel_spmd(nc, in_maps, list(range(NCORES)), trace=TRACE)
    LAST_EXEC_NS = res.exec_time_ns
    return np.concatenate([res.results[i]["y"] for i in range(NCORES)], axis=0)


# revision 21
# speedup vs baseline: 1.1126x; 1.1126x over previous
"""Sequence-parallel single-head attention block (LN -> QKV -> softmax(QK^T)V -> proj -> residual)
for 8 Trainium2 NeuronCores.

Sharding: core i owns query rows [1024*i, 1024*(i+1)). K/V projections over all 8192
tokens are replicated on every core (no collectives). Inside each core a flash-style
loop streams 512-key chunks: LN + K/V projection + scores + exp + AV accumulate,
with scores held transposed (keys on partitions) so that
  - exp() is the PSUM->SBUF eviction on the scalar engine,
  - softmax denominators come from a ones-vector matmul on the tensor engine,
  - P^T and V feed the AV matmul directly (no P transposes).

Host-side exact algebra folds:
  - ln_w/ln_b fold into the QKV weight matrices / biases,
  - the 1/sqrt(c) score scale folds into Wq/bq,
  - bk drops exactly (softmax is invariant to per-query score shifts),
  - bv folds into the output-projection bias (P @ 1 = denom), bp' = bp + Wp @ bv,
  - softmax runs without max subtraction (scores bounded ~|2| for these inputs).

Matmuls run in float32r (full PE rate); operands are rounded to fp32r for free inside
the DVE/ACT evictions that produce them.
"""

import math
from contextlib import ExitStack

import numpy as np

import concourse.bass as bass
import concourse.bacc as bacc
import concourse.tile as tile
from concourse import mybir
from concourse.bass_utils import run_bass_kernel_spmd
from concourse.masks import make_identity

N, NF = 8192, 512
NCORES = 8
BLK = N // NCORES          # 1024 query rows per core
MC = 512                   # key-chunk size
NCHUNK = N // MC           # 16
EPS = 1e-5

F32 = mybir.dt.float32
F32R = mybir.dt.float32r
AF = mybir.ActivationFunctionType

TRACE = False              # test.py flips this for timed runs
LAST_EXEC_NS = None

_cached_nc = None


def _build():
    nc = bacc.Bacc("TRN2", target_bir_lowering=False, debug=False)

    x_all = nc.dram_tensor("x_all", [N, NF], F32, kind="ExternalInput")
    xq = nc.dram_tensor("xq", [BLK, NF], F32, kind="ExternalInput")
    wqt = nc.dram_tensor("wqt", [NF, NF], F32, kind="ExternalInput")  # (Wq*ln_w).T * scale
    wkt = nc.dram_tensor("wkt", [NF, NF], F32, kind="ExternalInput")  # (Wk*ln_w).T
    wvt = nc.dram_tensor("wvt", [NF, NF], F32, kind="ExternalInput")  # (Wv*ln_w).T
    wpt = nc.dram_tensor("wpt", [NF, NF], F32, kind="ExternalInput")  # Wp.T
    bqs = nc.dram_tensor("bqs", [NF], F32, kind="ExternalInput")      # (bq + Wq@ln_b)*scale
    bp2 = nc.dram_tensor("bp2", [NF], F32, kind="ExternalInput")      # bp + Wp@bv
    y_out = nc.dram_tensor("y", [BLK, NF], F32, kind="ExternalOutput")

    with tile.TileContext(nc) as tc, ExitStack() as ctx:
        # ---- pools ----
        const = ctx.enter_context(tc.tile_pool(name="const", bufs=1))
        wpool = ctx.enter_context(tc.tile_pool(name="wpool", bufs=1))
        xcp = ctx.enter_context(tc.tile_pool(name="xcp", bufs=2))
        xtp = ctx.enter_context(tc.tile_pool(name="xtp", bufs=2))
        ktp = ctx.enter_context(tc.tile_pool(name="ktp", bufs=2))
        vp = ctx.enter_context(tc.tile_pool(name="vp", bufs=2))
        ptp = ctx.enter_context(tc.tile_pool(name="ptp", bufs=2))
        stat = ctx.enter_context(tc.tile_pool(name="stat", bufs=4))
        acc = ctx.enter_context(tc.tile_pool(name="acc", bufs=1))
        yp = ctx.enter_context(tc.tile_pool(name="yp", bufs=2))
        xop = ctx.enter_context(tc.tile_pool(name="xop", bufs=1))
        ps = ctx.enter_context(tc.tile_pool(name="ps", bufs=4, space="PSUM"))
        psav = ctx.enter_context(tc.tile_pool(name="psav", bufs=2, space="PSUM"))
        psd = ctx.enter_context(tc.tile_pool(name="psd", bufs=1, space="PSUM"))

        # ---- constants / weights ----
        ident_f = const.tile([128, 128], F32, tag="ident_f")
        make_identity(nc, ident_f[:])
        ident = const.tile([128, 128], F32R, tag="ident")
        nc.vector.tensor_copy(out=ident[:], in_=ident_f[:])
        ones_f = const.tile([128, MC], F32, tag="ones_f")
        nc.vector.memset(ones_f[:], 1.0)
        ones_col = const.tile([128, 1], F32R, tag="ones_col")
        nc.vector.tensor_copy(out=ones_col[:], in_=ones_f[:, 0:1])
        ones_row = const.tile([1, MC], F32R, tag="ones_row")
        nc.vector.tensor_copy(out=ones_row[:], in_=ones_f[0:1, :])
        eps_t = const.tile([128, 1], F32, tag="eps")
        nc.vector.memset(eps_t[:], EPS)
        ones11 = const.tile([1, 1], F32, tag="ones11")
        nc.vector.memset(ones11[:], 1.0)

        w_sb = {}
        for name, drm in (("wq", wqt), ("wk", wkt), ("wv", wvt), ("wp", wpt)):
            t = wpool.tile([128, 4, NF], F32R, tag=name)
            nc.gpsimd.dma_start(
                out=t[:], in_=drm.ap().rearrange("(s p) e -> p s e", p=128)
            )
            w_sb[name] = t
        bq_sb = const.tile([1, NF], F32R, tag="bq")
        nc.gpsimd.dma_start(out=bq_sb[:], in_=bqs.ap().rearrange("(o e) -> o e", o=1))
        bp2_sb = const.tile([128, NF], F32, tag="bp2")
        bp2_b = bass.AP(tensor=bp2.ap().tensor, offset=bp2.ap().offset,
                        ap=[[0, 128]] + bp2.ap().ap)
        nc.gpsimd.dma_start(out=bp2_sb[:], in_=bp2_b)

        qt_sb = acc.tile([128, 4, BLK], F32R, tag="qt")       # q^T [c, n]
        at_sb = acc.tile([128, 4, BLK], F32, tag="attnT")     # attn^T accumulator [c, n]
        den_sb = acc.tile([1, BLK], F32, tag="den")
        rd_sb = acc.tile([128, BLK // 128], F32, tag="rd")

        # ---- Phase 0: LayerNorm stats for all chunks (batches Ln/Exp into one
        # pair so the ACT table set never switches inside the main loop) ----
        NSTAT = NCHUNK * 4 + (BLK // 128)          # 64 x_all tiles + 8 xq tiles
        mv_all = acc.tile([128, NSTAT, 2], F32, tag="mv_all")
        rstd_all = acc.tile([128, NSTAT], F32, tag="rstd_all")
        for ch in range(NCHUNK + BLK // MC):
            if ch < NCHUNK:
                src, m0, sidx = x_all, ch * MC, ch * 4
            else:
                oc = ch - NCHUNK
                src, m0, sidx = xq, oc * MC, NCHUNK * 4 + oc * 4
            x0 = xcp.tile([128, 4, NF], F32, tag="xc")
            nc.sync.dma_start(
                out=x0[:],
                in_=src.ap()[m0:m0 + MC, :].rearrange("(t p) d -> p t d", p=128),
            )
            for t in range(4):
                st = stat.tile([128, 6], F32, tag="st")
                nc.vector.bn_stats(out=st[:], in_=x0[:, t, :])
                nc.vector.bn_aggr(out=mv_all[:, sidx + t, :], in_=st[:])
        # rstd = exp(-0.5*ln(var+eps)) for all tiles at once
        nc.scalar.activation(out=rstd_all[:], in_=mv_all[:, :, 1], func=AF.Ln,
                             bias=eps_t[:], scale=1.0)
        nc.scalar.activation(out=rstd_all[:], in_=rstd_all[:], func=AF.Exp,
                             scale=-0.5)

        def ln_chunk(src_dram, m0, sidx):
            """Load 512 rows starting at m0 (cast to f32r in-DMA), normalize with
            phase-0 stats. Returns [128, 4, 512] f32r (m within block, block, d)."""
            xc = xcp.tile([128, 4, NF], F32R, tag="xc")
            nc.gpsimd.dma_start(
                out=xc[:],
                in_=src_dram.ap()[m0:m0 + MC, :].rearrange("(t p) d -> p t d", p=128),
            )
            for t in range(4):
                nc.vector.tensor_scalar(
                    out=xc[:, t, :], in0=xc[:, t, :],
                    scalar1=mv_all[:, sidx + t, 0:1],
                    scalar2=rstd_all[:, sidx + t:sidx + t + 1],
                    op0=mybir.AluOpType.subtract, op1=mybir.AluOpType.mult,
                )
            return xc

        def transpose_chunk(xc):
            """[128,4,512] normalized chunk -> x^T chunk [128, 4, 512] = (d, dslice, m)."""
            xt = xtp.tile([128, 4, MC], F32R, tag="xt")
            for ds in range(4):
                ptile = ps.tile([128, MC], F32R, tag="ps")
                for t in range(4):
                    nc.tensor.transpose(
                        ptile[:, t * 128:(t + 1) * 128],
                        xc[:, t, ds * 128:(ds + 1) * 128],
                        ident[:],
                    )
                nc.scalar.activation(out=xt[:, ds, :], in_=ptile[:], func=AF.Copy)
            return xt

        # ---- Phase A: q^T for this core's 1024 rows ----
        for oc in range(BLK // MC):
            xc = ln_chunk(xq, oc * MC, NCHUNK * 4 + oc * 4)
            xt = transpose_chunk(xc)
            for cc in range(4):
                ptile = ps.tile([128, MC], F32, tag="ps")
                for ds in range(4):
                    nc.tensor.matmul(
                        ptile[:], w_sb["wq"][:, ds, cc * 128:(cc + 1) * 128],
                        xt[:, ds, :], start=(ds == 0), stop=False,
                    )
                nc.tensor.matmul(
                    ptile[:], bq_sb[:, cc * 128:(cc + 1) * 128], ones_row[:],
                    start=False, stop=True,
                )
                nc.scalar.activation(out=qt_sb[:, cc, oc * MC:(oc + 1) * MC],
                                     in_=ptile[:], func=AF.Copy)

        # ---- persistent denominator PSUM tiles ----
        pd = []
        for nh in range(2):
            pd_t = psd.tile([1, MC], F32, tag=f"d{nh}")
            pd.append(pd_t)

        # ---- Phase B: stream key chunks ----
        for ch in range(NCHUNK):
            xc = ln_chunk(x_all, ch * MC, ch * 4)
            xt = transpose_chunk(xc)

            # k^T chunk [c, m]
            kt = ktp.tile([128, 4, MC], F32R, tag="kt")
            for cc in range(4):
                ptile = ps.tile([128, MC], F32, tag="ps")
                for ds in range(4):
                    nc.tensor.matmul(
                        ptile[:], w_sb["wk"][:, ds, cc * 128:(cc + 1) * 128],
                        xt[:, ds, :], start=(ds == 0), stop=(ds == 3),
                    )
                nc.scalar.activation(out=kt[:, cc, :], in_=ptile[:], func=AF.Copy)

            # v chunk [m, c]
            vt = vp.tile([128, 4, NF], F32R, tag="vt")
            for mb in range(4):
                ptile = ps.tile([128, NF], F32, tag="ps")
                for ds in range(4):
                    nc.tensor.matmul(
                        ptile[:], xt[:, ds, mb * 128:(mb + 1) * 128],
                        w_sb["wv"][:, ds, :], start=(ds == 0), stop=(ds == 3),
                    )
                nc.scalar.activation(out=vt[:, mb, :], in_=ptile[:], func=AF.Copy)

            # scores^T -> exp -> P^T  [m, n]
            pt = ptp.tile([128, 4, BLK], F32R, tag="pt")
            for mb in range(4):
                for nh in range(2):
                    ptile = ps.tile([128, MC], F32, tag="ps")
                    for cc in range(4):
                        nc.tensor.matmul(
                            ptile[:], kt[:, cc, mb * 128:(mb + 1) * 128],
                            qt_sb[:, cc, nh * 512:(nh + 1) * 512],
                            start=(cc == 0), stop=(cc == 3),
                        )
                    nc.scalar.activation(
                        out=pt[:, mb, nh * 512:(nh + 1) * 512], in_=ptile[:],
                        func=AF.Exp,
                    )

            # denominators: ones^T @ P^T accumulated across all chunks
            for mb in range(4):
                for nh in range(2):
                    nc.tensor.matmul(
                        pd[nh][:], ones_col[:],
                        pt[:, mb, nh * 512:(nh + 1) * 512],
                        start=(ch == 0 and mb == 0), stop=(ch == NCHUNK - 1 and mb == 3),
                        skip_group_check=True,
                    )

            # attn^T partial: v^T-chunks as stationary, P^T moving
            for cc in range(4):
                for nh in range(2):
                    av = psav.tile([128, MC], F32, tag="av")
                    for mb in range(4):
                        nc.tensor.matmul(
                            av[:], vt[:, mb, cc * 128:(cc + 1) * 128],
                            pt[:, mb, nh * 512:(nh + 1) * 512],
                            start=(mb == 0), stop=(mb == 3),
                        )
                    dst = at_sb[:, cc, nh * 512:(nh + 1) * 512]
                    if ch == 0:
                        nc.vector.tensor_copy(out=dst, in_=av[:])
                    else:
                        nc.vector.tensor_tensor(
                            out=dst, in0=dst, in1=av[:], op=mybir.AluOpType.add,
                        )

        # ---- epilogue ----
        for nh in range(2):
            nc.vector.tensor_copy(out=den_sb[:, nh * 512:(nh + 1) * 512], in_=pd[nh][:])
        # transpose the denominator row into partitions: [1,128] x [1,1] matmuls
        prd = ps.tile([128, BLK // 128], F32, tag="ps")
        for j in range(BLK // 128):
            nc.tensor.matmul(prd[:, j:j + 1], den_sb[:, j * 128:(j + 1) * 128],
                             ones11[:], start=True, stop=True,
                             skip_group_check=True)
        nc.vector.reciprocal(out=rd_sb[:], in_=prd[:])

        # residual rows + folded bias
        xo_tiles = []
        for j in range(BLK // 128):
            xo = xop.tile([128, NF], F32, tag=f"xo{j}")
            nc.sync.dma_start(out=xo[:], in_=xq.ap()[j * 128:(j + 1) * 128, :])
            nc.vector.tensor_tensor(out=xo[:], in0=xo[:], in1=bp2_sb[:],
                                    op=mybir.AluOpType.add)
            xo_tiles.append(xo)

        at_rt = ptp.tile([128, 4, BLK], F32R, tag="pt")
        nc.vector.tensor_copy(out=at_rt[:], in_=at_sb[:])
        at_r = at_rt[:]
        for j in range(BLK // 128):
            ptile = ps.tile([128, NF], F32, tag="ps")
            for cc in range(4):
                nc.tensor.matmul(
                    ptile[:], at_r[:, cc, j * 128:(j + 1) * 128],
                    w_sb["wp"][:, cc, :], start=(cc == 0), stop=(cc == 3),
                )
            yt = yp.tile([128, NF], F32, tag="yt")
            nc.vector.tensor_scalar_mul(out=yt[:], in0=ptile[:],
                                        scalar1=rd_sb[:, j:j + 1])
            nc.vector.tensor_tensor(out=yt[:], in0=yt[:], in1=xo_tiles[j][:],
                                    op=mybir.AluOpType.add)
            nc.sync.dma_start(out=y_out.ap()[j * 128:(j + 1) * 128, :], in_=yt[:])

    nc.compile()
    return nc


def kernel(x, ln_w, ln_b, Wq, bq, Wk, bk, Wv, bv, Wp, bp):
    global _cached_nc, LAST_EXEC_NS
    x = np.ascontiguousarray(np.asarray(x, dtype=np.float32))
    ln_w = np.asarray(ln_w, np.float32)
    ln_b = np.asarray(ln_b, np.float32)
    Wq = np.asarray(Wq, np.float32)
    Wk = np.asarray(Wk, np.float32)
    Wv = np.asarray(Wv, np.float32)
    Wp = np.asarray(Wp, np.float32)
    scale = np.float32(1.0 / math.sqrt(NF))

    # exact algebraic folds (see module docstring)
    wqt = np.ascontiguousarray(((Wq * ln_w[None, :]).T * scale).astype(np.float32))
    wkt = np.ascontiguousarray((Wk * ln_w[None, :]).T.astype(np.float32))
    wvt = np.ascontiguousarray((Wv * ln_w[None, :]).T.astype(np.float32))
    wpt = np.ascontiguousarray(Wp.T.astype(np.float32))
    bqs = ((np.asarray(bq, np.float32) + Wq @ ln_b) * scale).astype(np.float32)
    bp2 = (np.asarray(bp, np.float32) + Wp @ np.asarray(bv, np.float32)).astype(np.float32)

    if _cached_nc is None:
        _cached_nc = _build()
    nc = _cached_nc

    in_maps = []
    for i in range(NCORES):
        in_maps.append({
            "x_all": x,
            "xq": np.ascontiguousarray(x[i * BLK:(i + 1) * BLK]),
            "wqt": wqt, "wkt": wkt, "wvt": wvt, "wpt": wpt,
            "bqs": bqs, "bp2": bp2,
        })
    res = run_bass_kernel_spmd(nc, in_maps, list(range(NCORES)), trace=TRACE)
    LAST_EXEC_NS = res.exec_time_ns
    return np.concatenate([res.results[i]["y"] for i in range(NCORES)], axis=0)
